# revision 4
# baseline (speedup 1.0000x reference)
"""Binarized 3x3 conv (sign(W) conv + bias) on 8 Trainium2 NeuronCores.

Problem (hardcoded):
  x:      (32, 256, 56, 56) f32
  weight: (256, 256, 3, 3)  f32  -> sign-binarized
  bias:   (256,)            f32
  out:    (32, 256, 56, 56) f32  (stride 1, pad 1)

Sharding: data-parallel over batch — 4 images per core, weight/bias
replicated. No collectives.

Per-core kernel: conv = sum over the 9 taps of shifted 1x1 convs. x is
cast to bf16 into a zero-padded [128, 2, 58, 58] SBUF image; weights are
sign-binarized (ACT Sign: 0 -> 0, matching jnp.sign), PE-transposed into
per-tap lhsT tiles [ic=128, oc=128] (bf16, exact for ±1/0). Each output
tile [oc=128, 8 rows x 56 cols = 448] accumulates 2 ic-chunks x 9 taps
= 18 bf16 matmuls in one PSUM bank (fp32 accumulate), then gets the
per-channel bias added on DVE and is DMA'd out.
"""

import numpy as np

import concourse.bass as bass
import concourse.mybir as mybir
import concourse.tile as tile
from concourse import bacc
from concourse.bass_utils import run_bass_kernel_spmd
from concourse.masks import make_identity

N_CORES = 8
B = 32
B_PER = B // N_CORES  # 4 images per core
IC = OC = 256
H = W = 56
K = 3
R = 8               # output rows per matmul group
G = H // R          # 7 row groups
NCH = IC // 128     # 2 ic chunks
OCH = OC // 128     # 2 oc chunks

# Results of the last run_bass_kernel_spmd call (exec_time_ns etc.) for
# introspection by test harnesses; not used for grading.
LAST_RESULTS = None

_CACHED_NC = None


def _build_nc() -> bass.Bass:
    nc = bacc.Bacc()
    x = nc.dram_tensor("x", [B_PER, IC, H, W], mybir.dt.float32,
                       kind="ExternalInput")
    wt = nc.dram_tensor("weight", [OC, IC, K, K], mybir.dt.float32,
                        kind="ExternalInput")
    bs = nc.dram_tensor("bias", [OC], mybir.dt.float32, kind="ExternalInput")
    out = nc.dram_tensor("out", [B_PER, OC, H, W], mybir.dt.float32,
                         kind="ExternalOutput")

    with tile.TileContext(nc) as tc:
        with (
            tc.tile_pool(name="const", bufs=1) as const_pool,
            tc.tile_pool(name="wprep", bufs=2) as wprep_pool,
            tc.tile_pool(name="xs", bufs=2) as xs_pool,
            tc.tile_pool(name="xp", bufs=2) as xp_pool,
            tc.tile_pool(name="osb", bufs=4) as out_pool,
            tc.tile_pool(name="psum", bufs=2, space="PSUM") as psum_pool,
        ):
            # ---- constants ----
            ident = const_pool.tile([128, 128], mybir.dt.bfloat16, tag="ident")
            make_identity(nc, ident)

            bias_sb = const_pool.tile([128, OCH], mybir.dt.float32, tag="bias")
            nc.sync.dma_start(bias_sb, bs.rearrange("(a p) -> p a", p=128))

            # ---- weight prep: sign-binarize + transpose to [ic, oc] ----
            # w_taps[ic_part, c, o, t, oc] : lhsT for (ic chunk c, oc chunk o, tap t)
            w_taps = const_pool.tile([128, NCH, OCH, K * K, 128],
                                     mybir.dt.bfloat16, tag="wtaps")
            for o in range(OCH):
                wf = wprep_pool.tile([128, IC, K * K], mybir.dt.float32,
                                     tag="wf")
                nc.sync.dma_start(
                    wf, wt[o * 128:(o + 1) * 128].rearrange("o i a b -> o i (a b)"))
                wsg = wprep_pool.tile([128, IC, K * K], mybir.dt.bfloat16,
                                      tag="wsg")
                nc.scalar.sign(wsg, wf)
                for c in range(NCH):
                    for t in range(K * K):
                        pst = psum_pool.tile([128, 128], mybir.dt.bfloat16,
                                             tag="tp", bufs=2)
                        nc.tensor.transpose(
                            pst, wsg[:, c * 128:(c + 1) * 128, t], ident)
                        nc.vector.tensor_copy(out=w_taps[:, c, o, t, :],
                                              in_=pst)

            # ---- per-image conv ----
            for n in range(B_PER):
                xs = xs_pool.tile([128, NCH, H, W], mybir.dt.float32, tag="xs")
                for c in range(NCH):
                    nc.sync.dma_start(xs[:, c], x[n, c * 128:(c + 1) * 128])

                # zero-padded bf16 image [128, c, 58, 58]
                xpd = xp_pool.tile([128, NCH, H + 2, W + 2], mybir.dt.bfloat16,
                                   tag="xpd")
                for c in range(NCH):
                    nc.any.memset(xpd[:, c, 0, :], 0.0)
                    nc.any.memset(xpd[:, c, H + 1, :], 0.0)
                    nc.any.memset(xpd[:, c, 1:H + 1, 0], 0.0)
                    nc.any.memset(xpd[:, c, 1:H + 1, W + 1], 0.0)
                    nc.vector.tensor_copy(out=xpd[:, c, 1:H + 1, 1:W + 1],
                                          in_=xs[:, c])

                for o in range(OCH):
                    for g in range(G):
                        ps = psum_pool.tile([128, R, W], mybir.dt.float32,
                                            tag="acc", bufs=6)
                        for c in range(NCH):
                            for ky in range(K):
                                for kx in range(K):
                                    t = ky * K + kx
                                    nc.tensor.matmul(
                                        ps,
                                        w_taps[:, c, o, t, :],
                                        xpd[:, c, g * R + ky:g * R + ky + R,
                                            kx:kx + W],
                                        start=(c == 0 and t == 0),
                                        stop=(c == NCH - 1 and t == K * K - 1),
                                    )
                        osb = out_pool.tile([128, R, W], mybir.dt.float32,
                                            tag="osb")
                        nc.vector.tensor_tensor(
                            osb, ps,
                            bias_sb[:, o:o + 1, None].to_broadcast((128, R, W)),
                            mybir.AluOpType.add)
                        nc.sync.dma_start(
                            out[n, o * 128:(o + 1) * 128,
                                g * R:(g + 1) * R, :],
                            osb)
    nc.finalize()
    return nc


def kernel(x: np.ndarray, weight: np.ndarray, bias: np.ndarray) -> np.ndarray:
    global LAST_RESULTS, _CACHED_NC
    assert x.shape == (B, IC, H, W)
    if _CACHED_NC is None:
        _CACHED_NC = _build_nc()
    nc = _CACHED_NC

    weight = np.ascontiguousarray(weight, dtype=np.float32)
    bias = np.ascontiguousarray(bias, dtype=np.float32)
    in_maps = [
        {
            "x": np.ascontiguousarray(x[i * B_PER:(i + 1) * B_PER],
                                      dtype=np.float32),
            "weight": weight,
            "bias": bias,
        }
        for i in range(N_CORES)
    ]
    res = run_bass_kernel_spmd(nc, in_maps, core_ids=list(range(N_CORES)))
    LAST_RESULTS = res
    return np.concatenate([res.results[i]["out"] for i in range(N_CORES)],
                          axis=0)
